# revision 19
# baseline (speedup 1.0000x reference)
"""EqPBCNN (perturbation-based nonlinearity compensation NN) Trainium2 Bass kernel.

Data-parallel over 8 NeuronCores: batch 65536 -> 8192 per core.

Math (per sample):
  G_(a,b) = sum_q x[a,q] * conj(x[b,q])      (pol-independent; pairs (a,b)=(n+L, m+n+L))
  h1[p,o] = sum_m x[m,p] * R[p,o,m],  R = sum_n W1'[p,o,(m,n)] * G
  h2 = CLrelu(h1) @ W2^T; E = CLrelu(h2) @ W3^T
  out = x[center,p] + E * 10^(task0/10)/2

v2 design: conjugate-canonical pairs (148 of 175; G_(b,a) = conj(G_(a,b)) folded
into the R weights with signs). Host pre-gathers the pair stacks into DRAM:
  SAr/SAi/SBr/SBi [296, B/8] bf16   rows = (q, off-diag pairs(127), q, diag(21))
  XPr/XPi [100, B/8] bf16           rows = (o, mi, p) x-replica for the T product
Device pipeline per chunk (NS=1024 cols):
  DVE products P1=SAr*SBr, P2=SAi*SBi (296 rows), P3=SAi*SBr, P4=SAr*SBi (254 rows)
  Gim = P3 - P4 on DVE; Gre = P1 + P2 folded into PE (P1,P2 fed separately)
  R matmuls (PE, bf16) -> Rre/Rim [100, NS] PSUM -> ACT copy to SBUF bf16
  T products U=XP*R (DVE bf16) -> final matmul (PE) -> h1 [8, NS]
  ACT lrelu / W2 / lrelu / W3 -> E [4, NS]; ACT exp for P; DVE E*P; Pool residual.
"""
import numpy as np
import ml_dtypes

BF16 = ml_dtypes.bfloat16

# ---------------- problem constants (hardcoded; must match reference) -------
BATCH = 65536
MT, LH = 41, 20          # filter taps, half window
NM = 2                   # modes / polarizations
H1, H2 = 2, 10
SLOPE = 0.01
NCORES = 8
BCORE = BATCH // NCORES  # 8192
NS = 1024               # columns per compute chunk (mega == chunk)
NCHUNK = BCORE // NS     # 8

# ---------------- triplet / canonical-pair tables ---------------------------
_idx = [(m, n) for m in range(-LH, LH + 1) for n in range(-LH, LH + 1)
        if abs(m * n) <= LH and abs(m + n) <= LH and n >= m]
H = len(_idx)            # 175
SYM = np.where(np.array([m for m, n in _idx]) != np.array([n for m, n in _idx]),
               2.0, 1.0).astype(np.float32)
M_VALS = sorted(set(m for m, n in _idx))     # 25 distinct m values
NMV = len(M_VALS)
M_POS = {m: i for i, m in enumerate(M_VALS)}
NO = H1 * NMV * NM       # 100 rows of R/T space: (o, mi, p)

# canonical pairs: key (a,b) a<=b; triplet h -> (pair index, Gim sign)
_ckeys = {}
_tripmap = []
for (m, n) in _idx:
    a, b = n + LH, m + n + LH
    key, s = ((a, b), 1.0) if a <= b else ((b, a), -1.0)
    _ckeys[key] = None
    _tripmap.append((key, s))
POFF = sorted([k for k in _ckeys if k[0] < k[1]], key=lambda k: (k[1] - k[0], k[0]))
PDIAG = sorted([k for k in _ckeys if k[0] == k[1]])
NOFF, NDIAG = len(POFF), len(PDIAG)          # 127, 21
NPAIR = NOFF + NDIAG                          # 148
# stack rows: q0-off(127), q1-off(127), q0-diag(21), q1-diag(21)
NROWS_RE = 2 * NPAIR                          # 296 (P1/P2/Gre rows)
NROWS_IM = 2 * NOFF                           # 254 (P3/P4/Gim rows)
_pairpos = {}
for i, k in enumerate(POFF):
    _pairpos[k] = ('off', i)
for i, k in enumerate(PDIAG):
    _pairpos[k] = ('diag', i)


def _stack_row(kind, i, q):
    return q * NOFF + i if kind == 'off' else NROWS_IM + q * NDIAG + i


def _orow(o, mi, p):
    return (o * NMV + mi) * NM + p


def _hrow(p, o, comp):
    return (p * H1 + o) * 2 + comp


def _h2row(p, q, comp):
    return (p * H2 + q) * 2 + comp


# split boundaries of the 296-row stacks
KSP_RE = [(0, 128), (128, 128), (256, NROWS_RE - 256)]   # 128,128,40
KSP_IM = [(0, 128), (128, NROWS_IM - 128)]               # 128,126


def build_static():
    """Gather row tables (host side) + final/contraction constants."""
    # source rows within xq82 [2*41, BCORE] (rows = q*41 + tap)
    a_src = np.zeros(NROWS_RE, np.int64)
    b_src = np.zeros(NROWS_RE, np.int64)
    for key in POFF + PDIAG:
        kind, i = _pairpos[key]
        a, b = key
        for q in range(NM):
            r = _stack_row(kind, i, q)
            a_src[r] = q * MT + a
            b_src[r] = q * MT + b
    # xrep rows (o, mi, p) -> tap m+L, pol p
    xp_src = np.zeros(NO, np.int64)
    for o in range(H1):
        for mi, mv in enumerate(M_VALS):
            for p in range(NM):
                xp_src[_orow(o, mi, p)] = p * MT + (mv + LH)
    # final contraction [100, 32]: 8-col groups for U1(+re) U2(-re) U3(+im) U4(+im)
    FINW = np.zeros((NO, 32), np.float32)
    for o in range(H1):
        for mi in range(NMV):
            for p in range(NM):
                r = _orow(o, mi, p)
                FINW[r, 0 + _hrow(p, o, 0)] = 1.0
                FINW[r, 8 + _hrow(p, o, 0)] = -1.0
                FINW[r, 16 + _hrow(p, o, 1)] = 1.0
                FINW[r, 24 + _hrow(p, o, 1)] = 1.0
    return {"a_src": a_src, "b_src": b_src, "xp_src": xp_src, "FINW": FINW}


def fold_weights(W1r, W1i, W2r, W2i, W3r, W3i):
    """Fold W1 (with SYM, pol-sum dup, conj-pair signs) into R-matmul lhsT."""
    Wr = W1r * SYM[None, None, :]   # [p, o, h]
    Wi = W1i * SYM[None, None, :]
    # WG [296, 200]: cols 0:100 -> Rre (+Wr), 100:200 -> Rim (+Wi); fed by P1 AND P2
    WG = np.zeros((NROWS_RE, 2 * NO), np.float32)
    # WI [254, 200]: Gim rows; cols 0:100 -> Rre (-s*Wi), 100:200 -> Rim (+s*Wr)
    WI = np.zeros((NROWS_IM, 2 * NO), np.float32)
    for h, (mn, (key, s)) in enumerate(zip(_idx, _tripmap)):
        m, n = mn
        kind, i = _pairpos[key]
        mi = M_POS[m]
        for p in range(NM):
            for o in range(H1):
                c = _orow(o, mi, p)
                for q in range(NM):
                    r = _stack_row(kind, i, q)
                    WG[r, c] += Wr[p, o, h]
                    WG[r, NO + c] += Wi[p, o, h]
                    if kind == 'off':
                        WI[r, c] += -s * Wi[p, o, h]
                        WI[r, NO + c] += s * Wr[p, o, h]
    WGP = np.zeros((3, 128, 2 * NO), np.float32)
    for k, (r0, rk) in enumerate(KSP_RE):
        WGP[k, :rk, :] = WG[r0:r0 + rk, :]
    WIP = np.zeros((2, 128, 2 * NO), np.float32)
    for k, (r0, rk) in enumerate(KSP_IM):
        WIP[k, :rk, :] = WI[r0:r0 + rk, :]
    # W2 lhsT [8, 40] on h1 rows (p,o,comp)
    W2L = np.zeros((8, 2 * H2 * NM), np.float32)
    for p in range(NM):
        for q in range(H2):
            for o in range(H1):
                W2L[_hrow(p, o, 0), _h2row(p, q, 0)] += W2r[p, q, o]
                W2L[_hrow(p, o, 1), _h2row(p, q, 0)] -= W2i[p, q, o]
                W2L[_hrow(p, o, 0), _h2row(p, q, 1)] += W2i[p, q, o]
                W2L[_hrow(p, o, 1), _h2row(p, q, 1)] += W2r[p, q, o]
    # W3 lhsT [40, 4]: out rows [re_p0, re_p1, im_p0, im_p1]; 1/NM folded
    W3L = np.zeros((2 * H2 * NM, 4), np.float32)
    s3 = 1.0 / NM
    for p in range(NM):
        for q in range(H2):
            W3L[_h2row(p, q, 0), 0 + p] += W3r[p, 0, q] * s3
            W3L[_h2row(p, q, 1), 0 + p] -= W3i[p, 0, q] * s3
            W3L[_h2row(p, q, 0), 2 + p] += W3i[p, 0, q] * s3
            W3L[_h2row(p, q, 1), 2 + p] += W3r[p, 0, q] * s3
    return {"WGP": WGP, "WIP": WIP, "W2L": W2L, "W3L": W3L}


# ---------------------------------------------------------------------------
def build_nc(bcore=BCORE, lrelu_mode="act"):
    """Build the Bass program for one core processing `bcore` samples."""
    import concourse.bass as bass
    import concourse.bacc as bacc
    import concourse.mybir as mybir
    from concourse.tile import TileContext
    import bass_rust

    nchunk = bcore // NS
    assert nchunk * NS == bcore
    f32 = mybir.dt.float32
    bf16 = mybir.dt.bfloat16
    AF = bass_rust.ActivationFunctionType
    OP = mybir.AluOpType

    nc = bacc.Bacc(None, target_bir_lowering=False, debug=False)
    saR = nc.declare_dram_parameter("SAr", [NROWS_RE, bcore], bf16, isOutput=False)
    saI = nc.declare_dram_parameter("SAi", [NROWS_RE, bcore], bf16, isOutput=False)
    sbR = nc.declare_dram_parameter("SBr", [NROWS_RE, bcore], bf16, isOutput=False)
    sbI = nc.declare_dram_parameter("SBi", [NROWS_RE, bcore], bf16, isOutput=False)
    xpR = nc.declare_dram_parameter("XPr", [NO, bcore], bf16, isOutput=False)
    xpI = nc.declare_dram_parameter("XPi", [NO, bcore], bf16, isOutput=False)
    pexD = nc.declare_dram_parameter("PEX4", [4, bcore], bf16, isOutput=False)
    ctrD = nc.declare_dram_parameter("CTR", [4, bcore], f32, isOutput=False)
    wgD = nc.declare_dram_parameter("WGP", [3, 128, 2 * NO], f32, isOutput=False)
    wiD = nc.declare_dram_parameter("WIP", [2, 128, 2 * NO], f32, isOutput=False)
    finD = nc.declare_dram_parameter("FINW", [NO, 32], f32, isOutput=False)
    w2D = nc.declare_dram_parameter("W2L", [8, 40], f32, isOutput=False)
    w3D = nc.declare_dram_parameter("W3L", [40, 4], f32, isOutput=False)
    outD = nc.declare_dram_parameter("OUT", [4, bcore], f32, isOutput=True)

    with TileContext(nc) as tc:
        with (
            tc.tile_pool(name="consts", bufs=1) as cp,
            tc.tile_pool(name="mega", bufs=2) as mp,
            tc.tile_pool(name="small", bufs=4) as sp,
            tc.tile_pool(name="prod", bufs=2) as up,
            tc.tile_pool(name="tt", bufs=4) as tp,
            tc.tile_pool(name="psumr", bufs=6, space="PSUM") as ppr,
            tc.tile_pool(name="psumt", bufs=2, space="PSUM") as ppt,
        ):
            def const_tile(src_ap, name):
                t32 = cp.tile(list(src_ap.shape), f32, name=name + "_32")
                nc.scalar.dma_start(out=t32[:], in_=src_ap)
                tr = cp.tile(list(src_ap.shape), bf16, name=name)
                nc.vector.tensor_copy(tr[:], t32[:])
                return tr

            pex = cp.tile([4, bcore], bf16, name="pex")
            wg_sb, wi_sb = [None] * 3, [None] * 2
            fw = {}

            def emit_consts():
                for k in range(3):
                    wg_sb[k] = const_tile(wgD[k], f"wg{k}")
                for k in range(2):
                    wi_sb[k] = const_tile(wiD[k], f"wi{k}")
                fw["fin"] = const_tile(finD[:], "fin")
                fw["w2"] = const_tile(w2D[:], "w2")
                fw["w3"] = const_tile(w3D[:], "w3")



            HS = NS // 2         # 512-col half for PSUM/matmul stages
            LS = NS              # per-chunk DMA load granularity
            stage = {}           # c -> products etc (1024-col tiles)
            lstage = {}          # lc -> loaded stack tiles (2048-col)
            rstage = {}          # (c,h) -> psum R tiles

            def lrelu(dst, src_ap):
                if lrelu_mode == "act":
                    nc.scalar.activation(dst, src_ap, AF.Lrelu, alpha=SLOPE)
                else:
                    nc.vector.tensor_scalar_mul(dst, src_ap, SLOPE)
                    nc.vector.tensor_tensor(dst, dst, src_ap, op=OP.max)

            def load_block(lc):
                # one 2048-col DMA block = 2 compute chunks
                ls = slice(lc * LS, (lc + 1) * LS)
                sa_r, sa_i, sb_r, sb_i = [], [], [], []
                for k, (r0, rk) in enumerate(KSP_RE):
                    for nm_, src_, lst, eng in (
                        (f"sar{k}", saR, sa_r, nc.sync), (f"sai{k}", saI, sa_i, nc.sync),
                        (f"sbr{k}", sbR, sb_r, nc.scalar), (f"sbi{k}", sbI, sb_i, nc.scalar)):
                        if k == 2 and nm_.startswith("sb"):
                            lst.append(None)   # all-diagonal split: SB == SA
                            continue
                        t = mp.tile([rk, LS], bf16, tag=nm_)
                        eng.dma_start(out=t[:], in_=src_[r0:r0 + rk, ls])
                        lst.append(t)
                sb_r[2] = sa_r[2]
                sb_i[2] = sa_i[2]
                xp_r = mp.tile([NO, LS], bf16, tag="xpr")
                xp_i = mp.tile([NO, LS], bf16, tag="xpi")
                nc.sync.dma_start(out=xp_r[:], in_=xpR[:, ls])
                nc.sync.dma_start(out=xp_i[:], in_=xpI[:, ls])
                lstage[lc] = (sa_r, sa_i, sb_r, sb_i, xp_r, xp_i)

            def stage_a(c):
                sa_r, sa_i, sb_r, sb_i, xp_r0, xp_i0 = lstage[c]
                qs = slice(0, NS)

                # DVE products (bf16 SBUF x SBUF, 2x mode) on full 1024 cols
                p1, p2, p3 = [], [], []
                for k, (r0, rk) in enumerate(KSP_RE):
                    t1 = up.tile([rk, NS], bf16, tag=f"p1_{k}")
                    t2 = up.tile([rk, NS], bf16, tag=f"p2_{k}")
                    sbrk = sa_r[2] if k == 2 else sb_r[k]
                    sbik = sa_i[2] if k == 2 else sb_i[k]
                    nc.vector.tensor_tensor(t1[:], sa_r[k][:, qs], sbrk[:, qs], op=OP.mult)
                    nc.vector.tensor_tensor(t2[:], sa_i[k][:, qs], sbik[:, qs], op=OP.mult)
                    p1.append(t1)
                    p2.append(t2)
                for k, (r0, rk) in enumerate(KSP_IM):
                    t3 = up.tile([rk, NS], bf16, tag=f"p3_{k}")
                    t4 = up.tile([rk, NS], bf16, tag=f"p4_{k}")
                    nc.vector.tensor_tensor(t3[:], sa_i[k][:rk, qs], sb_r[k][:rk, qs], op=OP.mult)
                    nc.vector.tensor_tensor(t4[:], sa_r[k][:rk, qs], sb_i[k][:rk, qs], op=OP.mult)
                    nc.vector.tensor_tensor(t3[:], t3[:], t4[:], op=OP.subtract)
                    p3.append(t3)
                stage[c] = (p1, p2, p3, xp_r0, xp_i0)

            def stage_b(c, h):
                # R matmuls for one 512-col half
                p1, p2, p3, xp_r, xp_i = stage[c]
                hs = slice(h * HS, (h + 1) * HS)
                p_rre = ppr.tile([128, HS], f32, tag="pr")
                p_rim = ppr.tile([128, HS], f32, tag="pr")
                for k, (r0, rk) in enumerate(KSP_RE):
                    wg = wg_sb[k]
                    nc.tensor.matmul(p_rre[:NO], wg[:rk, 0:NO], p1[k][:, hs], start=(k == 0), stop=False)
                    nc.tensor.matmul(p_rre[:NO], wg[:rk, 0:NO], p2[k][:, hs], start=False, stop=False)
                    nc.tensor.matmul(p_rim[:NO], wg[:rk, NO:2 * NO], p1[k][:, hs], start=(k == 0), stop=False)
                    nc.tensor.matmul(p_rim[:NO], wg[:rk, NO:2 * NO], p2[k][:, hs], start=False, stop=False)
                for k, (r0, rk) in enumerate(KSP_IM):
                    wi = wi_sb[k]
                    nc.tensor.matmul(p_rre[:NO], wi[:rk, 0:NO], p3[k][:, hs], start=False, stop=(k == 1))
                    nc.tensor.matmul(p_rim[:NO], wi[:rk, NO:2 * NO], p3[k][:, hs], start=False, stop=(k == 1))
                rstage[(c, h)] = (p_rre, p_rim)

            def stage_c(c, h):
                p1, p2, p3, xp_r0, xp_i0 = stage[c]
                p_rre, p_rim = rstage.pop((c, h))
                hs = slice(h * HS, (h + 1) * HS)
                qh = slice(h * HS, (h + 1) * HS)
                cs2 = slice(c * NS + h * HS, c * NS + (h + 1) * HS)
                # T products: U = XP * R (R copied to SBUF bf16 first)
                rre_s = tp.tile([NO, HS], bf16, tag="rres")
                rim_s = tp.tile([NO, HS], bf16, tag="rims")
                nc.scalar.copy(rre_s[:], p_rre[:NO])
                nc.scalar.copy(rim_s[:], p_rim[:NO])
                u1 = tp.tile([NO, HS], bf16, tag="u1")
                u2 = tp.tile([NO, HS], bf16, tag="u2")
                u3 = tp.tile([NO, HS], bf16, tag="u3")
                u4 = tp.tile([NO, HS], bf16, tag="u4")
                nc.vector.tensor_tensor(u1[:], xp_r0[:, qh], rre_s[:], op=OP.mult)
                nc.vector.tensor_tensor(u2[:], xp_i0[:, qh], rim_s[:], op=OP.mult)
                nc.vector.tensor_tensor(u3[:], xp_r0[:, qh], rim_s[:], op=OP.mult)
                nc.vector.tensor_tensor(u4[:], xp_i0[:, qh], rre_s[:], op=OP.mult)

                # final contraction -> h1 [8, HS]
                fin_sb = fw["fin"]
                p_h1 = ppt.tile([8, HS], f32, tag="pt")
                nc.tensor.matmul(p_h1[:8], fin_sb[:, 0:8], u1[:], start=True, stop=False)
                nc.tensor.matmul(p_h1[:8], fin_sb[:, 8:16], u2[:], start=False, stop=False)
                nc.tensor.matmul(p_h1[:8], fin_sb[:, 16:24], u3[:], start=False, stop=False)
                nc.tensor.matmul(p_h1[:8], fin_sb[:, 24:32], u4[:], start=False, stop=True)

                # MLP tail
                h1s = tp.tile([8, HS], bf16, tag="h1s")
                lrelu(h1s[:], p_h1[:8])
                p_h2 = ppt.tile([40, HS], f32, tag="pt")
                nc.tensor.matmul(p_h2[:40], fw["w2"][:], h1s[:], start=True, stop=True)
                h2s = tp.tile([40, HS], bf16, tag="h2s")
                lrelu(h2s[:], p_h2[:40])
                p_e = ppt.tile([4, HS], f32, tag="pt")
                nc.tensor.matmul(p_e[:4], fw["w3"][:], h2s[:], start=True, stop=True)

                # OUT += E*P (OUT pre-filled with center taps)
                ep = tp.tile([4, HS], f32, tag="ep")
                nc.vector.tensor_tensor(ep[:], p_e[:4], pex[:, cs2], op=OP.mult)
                nc.gpsimd.dma_start(out=outD[:, cs2], in_=ep[:], accum_op=OP.add)

            # 3-stage software pipeline over halves; B(t) runs two R-blocks
            # ahead of C(t) so the tail latency never stalls the PE queue
            halves = [(c, h) for c in range(nchunk) for h in (0, 1)]
            load_block(0)
            # pre-fill OUT with the center taps; E*P accumulates onto it via
            # SWDGE (also absorbs the one-time Q7 SWDGE IRAM load here)
            nc.gpsimd.dma_start(out=outD[:, :], in_=ctrD[:, :])
            # P = 10^(t/10) precomputed on host
            nc.sync.dma_start(out=pex[:], in_=pexD[:, :])
            emit_consts()
            stage_a(0)
            stage_b(*halves[0])
            stage_b(*halves[1])
            for i, (c, h) in enumerate(halves):
                nx2 = halves[i + 2] if i + 2 < len(halves) else None
                if nx2 is not None:
                    if nx2[1] == 0 and nx2[0] < nchunk:
                        load_block(nx2[0])
                        stage_a(nx2[0])
                    stage_b(*nx2)
                stage_c(c, h)
                if h == 1:
                    stage.pop(c)
                    lstage.pop(c)
    nc.compile()
    return nc


# ---------------------------------------------------------------------------
def _prep_core_inputs(inputs, static, folded):
    """Host-side gather + shard. Returns list of per-core in_maps."""
    xr = np.asarray(inputs["x_real"])     # [B, 41, 2]
    xi = np.asarray(inputs["x_imag"])
    t0 = np.ascontiguousarray(np.asarray(inputs["task_info"])[:, 0])
    # xq82 rows = q*41 + tap
    xrq = np.ascontiguousarray(xr.transpose(2, 1, 0).reshape(2 * MT, BATCH))
    xiq = np.ascontiguousarray(xi.transpose(2, 1, 0).reshape(2 * MT, BATCH))
    a_src, b_src, xp_src = static["a_src"], static["b_src"], static["xp_src"]
    SAr = xrq[a_src].astype(BF16)
    SAi = xiq[a_src].astype(BF16)
    SBr = xrq[b_src].astype(BF16)
    SBi = xiq[b_src].astype(BF16)
    XPr = xrq[xp_src].astype(BF16)
    XPi = xiq[xp_src].astype(BF16)
    PEX4 = np.broadcast_to((10.0 ** (t0[None, :] / 10.0)).astype(np.float32),
                           (4, BATCH)).astype(BF16)
    CTR = np.stack([xrq[LH], xrq[MT + LH], xiq[LH], xiq[MT + LH]], axis=0)
    shared = {"WGP": folded["WGP"], "WIP": folded["WIP"], "FINW": static["FINW"],
              "W2L": folded["W2L"], "W3L": folded["W3L"]}
    in_maps = []
    for c in range(NCORES):
        s = slice(c * BCORE, (c + 1) * BCORE)
        m = dict(shared)
        m["SAr"] = np.ascontiguousarray(SAr[:, s])
        m["SAi"] = np.ascontiguousarray(SAi[:, s])
        m["SBr"] = np.ascontiguousarray(SBr[:, s])
        m["SBi"] = np.ascontiguousarray(SBi[:, s])
        m["XPr"] = np.ascontiguousarray(XPr[:, s])
        m["XPi"] = np.ascontiguousarray(XPi[:, s])
        m["PEX4"] = np.ascontiguousarray(PEX4[:, s])
        m["CTR"] = np.ascontiguousarray(CTR[:, s])
        in_maps.append(m)
    return in_maps


_CACHE = {}


def kernel(**inputs):
    from concourse.bass_utils import run_bass_kernel_spmd

    static = build_static()
    folded = fold_weights(
        np.asarray(inputs["W1_real"]), np.asarray(inputs["W1_imag"]),
        np.asarray(inputs["W2_real"]), np.asarray(inputs["W2_imag"]),
        np.asarray(inputs["W3_real"]), np.asarray(inputs["W3_imag"]),
    )
    if "nc" not in _CACHE:
        _CACHE["nc"] = build_nc()
    nc = _CACHE["nc"]
    in_maps = _prep_core_inputs(inputs, static, folded)
    res = run_bass_kernel_spmd(nc, in_maps, list(range(NCORES)))
    out = np.empty((BATCH, NM, 2), np.float32)
    for c in range(NCORES):
        o4 = res.results[c]["OUT"]
        s = slice(c * BCORE, (c + 1) * BCORE)
        out[s, 0, 0] = o4[0]
        out[s, 1, 0] = o4[1]
        out[s, 0, 1] = o4[2]
        out[s, 1, 1] = o4[3]
    return out
